# revision 9
# baseline (speedup 1.0000x reference)
"""Trainium2 Bass kernel for vector-neuron multi-head attention, v2.2.

Sharding: 8 cores = 4 batches x 2 head-groups (tensor parallel). Each core
projects q/k/z for its 4 heads (128 of 256 output channels) over the full
M=N=2048 tokens, runs attention for those heads, and computes a PARTIAL
final Wo projection (contraction over its 128 channels only). The host
sums the two partials per batch and adds the Wo bias.

Layout tricks:
  - wq/wk columns are host-PERMUTED (ch-major, head-minor) so the per-head
    (d,ch)-packed qf/kf tiles are natural-partition-order fanout DMAs from
    the projection output ([128, chunk] -> [32, 4h, chunk]).
  - wz keeps the original head-contiguous layout; the AV operand zfts
    [n, (d*32+ch)+ones] is built by PE transposes DIRECTLY from the z
    projection (32-row tiles at base partition 32h via tile_position).
  - Wo rows get the q/k permutation so the reverse gather DMA feeds the
    final matmul directly. y partials are written bf16 (summed on host).

Scheduling: every engine queue is in-order, so cross-step overlap must be
EMITTED interleaved. All non-attention work (next rep's input loads,
projections, fanouts, transposes; this rep's per-m-half final projection)
is chopped into generator pieces and dripped one per odd n-tile slot of
the attention blocks (8 blocks x 16 slots/rep). ACT then runs its 128
softmax exps (~133us/rep) nearly back-to-back, with PE (~136us/rep)
co-bound. ~30 DMAs/rep.
"""

from collections import deque
from contextlib import ExitStack

import numpy as np

import concourse.bacc as bacc
import concourse.bass as bass
import concourse.tile as tile
from concourse import mybir
from concourse.bass_utils import run_bass_kernel_spmd

FP32 = mybir.dt.float32
BF16 = mybir.dt.float16  # fp16: 10 mantissa bits, same PE speed as bf16
AF = mybir.ActivationFunctionType
ALU = mybir.AluOpType

EMB = 256
HEADS = 8
EPS = 1e-6
B = 4
N = 2048          # tokens (M = N here)
HL = 4            # heads per core
CH = 32           # channels per head
SCALE = 1.0 / np.sqrt(3.0 * CH)
NT = N // 128     # 16 n-tiles
P = 128


def ts(i, s):
    return slice(i * s, (i + 1) * s)


def build_nc(nrep=1):
    nc = bacc.Bacc("TRN2", target_bir_lowering=False, debug=False)

    xs = {
        t: nc.dram_tensor(f"x{t}", [EMB, 3, N], BF16, kind="ExternalInput").ap()
        for t in ("q", "k", "z")
    }
    ws = {
        t: nc.dram_tensor(f"w{t}", [EMB, P], BF16, kind="ExternalInput").ap()
        for t in ("q", "k", "z")
    }
    wo = nc.dram_tensor("wo", [P, EMB], BF16, kind="ExternalInput").ap()
    us = {
        t: nc.dram_tensor(f"u{t}", [P, 3], FP32, kind="ExternalInput").ap()
        for t in ("q", "k", "z")
    }
    ident = nc.dram_tensor("ident", [P, P], BF16, kind="ExternalInput").ap()
    y = nc.dram_tensor("y", [EMB, 3, N], BF16, kind="ExternalOutput").ap()

    xr = {t: x.rearrange("(c p) d t -> p c d t", p=P) for t, x in xs.items()}
    wr = {t: w.rearrange("(c p) e -> p c e", p=P) for t, w in ws.items()}
    yr = y.rearrange("(c p) d t -> p c d t", p=P)

    with tile.TileContext(nc) as tc:
        with ExitStack() as ctx:
            pool = lambda name, bufs, **kw: ctx.enter_context(
                tc.tile_pool(name=name, bufs=bufs, **kw)
            )
            consts = pool("consts", 1)
            xin_pool = pool("xin", 2)
            proj_pool = pool("proj", 2)
            qf_pool = pool("qf", 2)
            kf_pool = pool("kf", 2)
            zft_pool = pool("zft", 2)
            ex_pool = pool("ex", 3)
            avsb_pool = pool("avsb", 2)
            inv_pool = pool("inv", 1)
            outh_pool = pool("outh", 1)
            outall_pool = pool("outall", 1)
            y_pool = pool("ysb", 2)
            pst_pool = pool("pst", 2, space="PSUM")
            pav_pool = pool("pav", 1, space="PSUM")
            pzt_pool = pool("pzt", 1, space="PSUM")
            pinv_pool = pool("pinv", 1, space="PSUM")

            # constants
            w_sb = {}
            u_sb = {}
            for t in ("q", "k", "z"):
                w_sb[t] = consts.tile([P, 2, P], BF16, tag=f"w{t}", name=f"w{t}_sb")
                nc.sync.dma_start(out=w_sb[t], in_=wr[t])
                u_sb[t] = consts.tile([P, 3], FP32, tag=f"u{t}", name=f"u{t}_sb")
                nc.sync.dma_start(out=u_sb[t], in_=us[t])
            wo_sb = consts.tile([P, EMB], BF16, tag="wo")
            nc.sync.dma_start(out=wo_sb, in_=wo)
            ident_sb = consts.tile([P, P], BF16, tag="ident")
            nc.sync.dma_start(out=ident_sb, in_=ident)
            ones96 = consts.tile([1, 96], BF16, tag="ones96")
            nc.vector.memset(ones96, 1.0)

            state = {}  # per-rep tiles, filled by preamble(rep)

            def preamble(rep):
                """Loads + projections + fanouts + z transposes for `rep`,
                yielded as one PE/DMA-sized piece per next()."""
                st_r = {}
                state[rep] = st_r
                xin = {}
                for t in ("q", "k"):
                    xin[t] = xin_pool.tile(
                        [P, 2, 3, N], BF16, tag="xin", name=f"x{t}in"
                    )
                    nc.sync.dma_start(out=xin[t], in_=xr[t])
                yield
                qf = st_r["qf"] = qf_pool.tile([96, HL, N], BF16, tag="qf", name="qf")
                kf = st_r["kf"] = kf_pool.tile([96, HL, N], BF16, tag="kf", name="kf")
                projs = {}
                for t in ("q", "k", "z"):
                    proj = proj_pool.tile([P, 3, N], BF16, tag="proj", name=f"p{t}")
                    projs[t] = proj
                    for d in range(3):
                        for nt in range(2):
                            ps = pst_pool.tile(
                                [P, 1024], FP32, tag="pst", name="projps"
                            )
                            for hf in range(2):
                                for cc in range(2):
                                    nc.tensor.matmul(
                                        ps[:, ts(hf, 512)],
                                        lhsT=w_sb[t][:, cc, :],
                                        rhs=xin[t][:, cc, d, ts(2 * nt + hf, 512)],
                                        start=(cc == 0),
                                        stop=(cc == 1),
                                    )
                            nc.vector.tensor_scalar_add(
                                proj[:, d, ts(nt, 1024)],
                                ps,
                                u_sb[t][:, d : d + 1],
                            )
                            yield
                        if t in ("q", "k"):
                            for c in range(2):
                                nc.sync.dma_start(
                                    out=st_r[t + "f"][ts(d, 32), :, ts(c, 1024)],
                                    in_=proj[:, d, ts(c, 1024)],
                                )
                    if t == "q":
                        # defer xz so its xin slot (shared with xq) is free
                        xin["z"] = xin_pool.tile(
                            [P, 2, 3, N], BF16, tag="xin", name="xzin"
                        )
                        nc.sync.dma_start(out=xin["z"], in_=xr["z"])
                        yield
                zfts = st_r["zfts"] = zft_pool.tile(
                    [P, HL, NT, 98], BF16, tag="zfts", name="zfts"
                )
                nc.vector.memset(
                    zfts.rearrange("p h n c -> p (h n) c")[:, :, 96:97], 1.0
                )
                pz = projs["z"]
                for nt in range(NT):
                    # one [128,128] transpose per (d, ntile) covers all 4
                    # heads: out cols = (h, ch); the DVE copy then splits
                    # them across the per-head zfts slices
                    zt = pzt_pool.tile([P, 3, P], BF16, tag="pzt", name="zt")
                    for d in range(3):
                        nc.tensor.transpose(
                            zt[:, d, :], pz[:, d, ts(nt, P)], ident_sb
                        )
                    nc.vector.tensor_copy(
                        zfts[:, :, nt, :96].rearrange("p h (d c) -> p h d c", c=32),
                        zt.rearrange("p d (h c) -> p h d c", c=32),
                    )
                    yield

            def final_proj_pieces(rep, mh):
                """Gather + partial Wo projection for one m-half."""
                st_r = state[rep]
                for d in range(3):
                    nc.sync.dma_start(
                        out=st_r["out_all"][:, d, ts(mh, 1024)],
                        in_=st_r["outh"][ts(d, 32), :, mh, :],
                    )
                yield
                yield
                yield
                for d in range(3):
                    for mt in range(2):
                        m0 = 1024 * mh + 512 * mt
                        ps = pst_pool.tile([P, 1024], FP32, tag="pst", name="yps")
                        for eo in range(2):
                            nc.tensor.matmul(
                                ps[:, ts(eo, 512)],
                                lhsT=wo_sb[:, ts(eo, P)],
                                rhs=st_r["out_all"][:, d, m0 : m0 + 512],
                                start=True,
                                stop=True,
                            )
                        yp = y_pool.tile([P, 2, 512], BF16, tag="ysb", name="yp")
                        nc.vector.tensor_copy(
                            yp, ps.rearrange("p (e m) -> p e m", e=2)
                        )
                        nc.sync.dma_start(out=yr[:, :, d, m0 : m0 + 512], in_=yp)
                        yield

            work = deque()

            def pump():
                while work:
                    try:
                        next(work[0])
                        return
                    except StopIteration:
                        work.popleft()

            def attention(rep):
                # safety: if the drip slots ran out, finish this rep's
                # preamble before consuming its tiles
                while "zfts" not in state.get(rep, {}) and work:
                    pump()
                st_r = state[rep]
                qf, kf, zfts = st_r["qf"], st_r["kf"], st_r["zfts"]
                st_r["out_all"] = outall_pool.tile([P, 3, N], BF16, tag="outall", name="out_all")
                st_r["outh"] = outh_pool.tile([96, HL, 2, 1024], BF16, tag="outh", name="outh")
                for mh in range(2):
                    for h in range(HL):
                        av = pav_pool.tile([97, 1024], FP32, tag="pav", name="av")

                        def av_accum(nt, ex, av=av, h=h):
                            for mc in range(2):
                                nc.tensor.matmul(
                                    av[:, ts(mc, 512)],
                                    lhsT=zfts[:, h, nt, :97],
                                    rhs=ex[:, ts(mc, 512)],
                                    start=(nt == 0),
                                    stop=(nt == NT - 1),
                                )

                        # AV trails scores by one tile: exp(nt)'s input is
                        # ready a full slot early, so drip-piece PE spikes
                        # eat into the cushion instead of stalling ACT
                        prev_ex = None
                        for nt in range(NT):
                            st = pst_pool.tile(
                                [P, 1024], FP32, tag="pst", name="st"
                            )
                            for mc in range(2):
                                nc.tensor.matmul(
                                    st[:, ts(mc, 512)],
                                    lhsT=kf[:, h, ts(nt, P)],
                                    rhs=qf[:, h, ts(2 * mh + mc, 512)],
                                    start=True,
                                    stop=True,
                                )
                            ex = ex_pool.tile([P, 1024], BF16, tag="ex", name="ex")
                            nc.scalar.activation(ex, st, AF.Exp, scale=float(SCALE))
                            if prev_ex is not None:
                                av_accum(nt - 1, prev_ex)
                            prev_ex = ex
                            if nt % 2 == 1:
                                pump()
                        av_accum(NT - 1, prev_ex)
                        # normalize: row 96 of av is the softmax denominator.
                        # recip reads the fp32 PSUM row; the bulk av evicts
                        # to bf16 (its 0.4% rounding averages out in Wo)
                        inv = inv_pool.tile([1, 1024], BF16, tag="inv", name="inv")
                        with nc.allow_low_precision(reason="softmax inv fp16"):
                            nc.vector.reciprocal(inv, av[96:97, :])
                        av_sb = avsb_pool.tile(
                            [96, 1024], BF16, tag="avsb", name="av_sb"
                        )
                        nc.vector.tensor_copy(av_sb, av[0:96, :])
                        for mc in range(2):
                            invb = pinv_pool.tile(
                                [96, 512], FP32, tag="pinv", name="invb"
                            )
                            nc.tensor.matmul(
                                invb,
                                lhsT=ones96,
                                rhs=inv[:, ts(mc, 512)],
                                start=True,
                                stop=True,
                            )
                            nc.vector.tensor_tensor(
                                st_r["outh"][:, h, mh, ts(mc, 512)],
                                av_sb[0:96, ts(mc, 512)],
                                invb,
                                ALU.mult,
                            )
                    work.append(final_proj_pieces(rep, mh))

            for piece in preamble(0):  # first rep: standalone preamble
                pass
            for rep in range(nrep):
                if rep + 1 < nrep:
                    work.append(preamble(rep + 1))
                attention(rep)
            while work:  # drain the last rep's final projection
                pump()
                if not work:
                    break

    nc.compile()
    return nc


_NC_CACHE = {}


def get_nc():
    if "nc" not in _NC_CACHE:
        _NC_CACHE["nc"] = build_nc()
    return _NC_CACHE["nc"]


def _perm_cols(w):
    # [256, 128] -> columns reordered ch-major, head-minor
    return np.ascontiguousarray(
        w.reshape(EMB, HL, CH).transpose(0, 2, 1).reshape(EMB, P)
    )


def _perm_rows(a):
    # [128, ...] -> rows reordered ch-major, head-minor
    s = a.shape
    return np.ascontiguousarray(
        a.reshape(HL, CH, *s[1:]).transpose(1, 0, *range(2, 1 + len(s))).reshape(s)
    )


def make_in_maps(Q, K, Z, Wq_w, Wq_b, Wk_w, Wk_b, Wz_w, Wz_b, Wo_w, Wo_b):
    bf16 = mybir.dt.np(BF16)

    def u_of(b):
        b = np.asarray(b, np.float32)
        return (EPS * b / np.linalg.norm(b, axis=1, keepdims=True)).astype(np.float32)

    uq, uk, uz = u_of(Wq_b), u_of(Wk_b), u_of(Wz_b)
    Qb = np.asarray(Q).astype(bf16)
    Kb = np.asarray(K).astype(bf16)
    Zb = np.asarray(Z).astype(bf16)
    Wqb = np.asarray(Wq_w, np.float32)
    Wkb = np.asarray(Wk_w, np.float32)
    Wzb = np.asarray(Wz_w, np.float32)
    Wob = np.asarray(Wo_w, np.float32)
    ident = np.eye(P, dtype=np.float32).astype(bf16)

    in_maps = []
    for core in range(8):
        b, g = core // 2, core % 2
        cols = slice(P * g, P * (g + 1))
        in_maps.append(
            {
                "xq": np.ascontiguousarray(Qb[b]),
                "xk": np.ascontiguousarray(Kb[b]),
                "xz": np.ascontiguousarray(Zb[b]),
                "wq": _perm_cols(Wqb[:, cols]).astype(bf16),
                "wk": _perm_cols(Wkb[:, cols]).astype(bf16),
                "wz": np.ascontiguousarray(Wzb[:, cols]).astype(bf16),
                "wo": _perm_rows(np.ascontiguousarray(Wob[cols, :])).astype(bf16),
                "uq": _perm_rows(uq[cols]),
                "uk": _perm_rows(uk[cols]),
                "uz": np.ascontiguousarray(uz[cols]),
                "ident": ident,
            }
        )
    return in_maps


def assemble(results, Wo_b=None):
    out = np.empty((B, EMB, 3, N), dtype=np.float32)
    for b in range(B):
        out[b] = results[2 * b]["y"].astype(np.float32) + results[2 * b + 1][
            "y"
        ].astype(np.float32)
    if Wo_b is not None:
        bo = np.asarray(Wo_b, np.float32)
        uo = EPS * bo / np.linalg.norm(bo, axis=1, keepdims=True)
        out += uo[None, :, :, None]
    return out


def kernel(**inputs):
    nc = get_nc()
    in_maps = make_in_maps(**inputs)
    res = run_bass_kernel_spmd(nc, in_maps, list(range(8)))
    return assemble(res.results, Wo_b=inputs["Wo_b"])


if __name__ == "__main__":
    nc = build_nc()
    print("built ok")
